# revision 19
# baseline (speedup 1.0000x reference)
"""Trainium2 Bass kernel for nn_AttentionV4 (patch attention, 8 heads on 8 cores).

Pipeline per core (= per head h), bf16 compute / fp32 accumulation:
  - The 1x1 qkv conv + depthwise 3x3 conv are fused into one dense 3x3 conv,
    expressed as a single matmul over a 6x6-windowed patch basis:
      Q/K/V[r, n] = sum_kappa W4[kappa, r] * Xp[kappa, n],
    kappa = (ph, pw, c) in [6,6,48] (1728, chunked 14 x 128), n = interior
    patch (64x64 grid = 4096; boundary patches of the stride-4 pad-4 unfold
    are exactly zero and are handled analytically). Xp chunks are DMA'd one
    per 128-kappa chunk via class-affine views of xb (each (dh,dw) class is
    an affine (hm, wm, c) box).
  - Head-matrix rows are ordered (khB, cl, kh1, kw) so each half of the fold
    rows (kh<2 vs kh>=2) is partition-contiguous; the output AllToAll splits
    into two halves, the second overlapping the first half's projection.
  - l2-normalize Q (x temperature) and K per column, A = Qn^T Kn in [-1,1],
    so softmax needs no max subtraction: E = exp(A), Z = rowsum(E) + 260
    (260 = number of zero boundary K columns, each contributing exp(0)).
  - out = (V/Z) @ E; attention is software-pipelined: QK strips of group g+1
    interleave with AV j-block pairs of group g so the PE never waits on exp.
"""
import sys
import types

sys.path.insert(0, "/opt/trn_rl_repo")

import numpy as np
import ml_dtypes

BF16 = ml_dtypes.bfloat16

# ---------------------------------------------------------------- constants
C = 48          # image channels
CH = 6          # channels per head
NH = 8          # heads == cores
GN = 64         # interior patch grid
N = GN * GN     # 4096 interior patches
M96 = 96        # rows of a head matrix (6ch * 4 * 4)
NKAP = 1728     # 36 windows * 48 channels
ZCORR = 260.0   # 4356 - 4096 zero K-columns, exp(0) each
NPIECE = 8      # front-end N pieces (8 patch rows, 512 patches each)
NCORES = 8
NCHUNK14 = 14

# group list (ph, pw) in kappa order
_GROUPS = []
for _dh, _dw in [(0, 0), (0, 1), (1, 0), (1, 1)]:
    for _hm in range(4 if _dh == 0 else 2):
        for _wm in range(4 if _dw == 0 else 2):
            _GROUPS.append((_dh * 4 + _hm, _dw * 4 + _wm))

# (dh, dw) class of each 128-kappa chunk (class boundaries at 768/1152/1536;
# chunk 13's pad rows 1728..1792 are genuine zeros, so its class is harmless)
def _chunk_plan14():
    cls_edges = [(0, (0, 0)), (768, (0, 1)), (1152, (1, 0)), (1536, (1, 1))]
    plan = []
    for k in range(NCHUNK14):
        k0 = 128 * k
        dh, dw = [c for e, c in cls_edges if e <= k0][-1]
        plan.append((dh, dw))
    return plan

CHUNKS14 = _chunk_plan14()

# kappa order (ph, pw, c), c fastest within each group
_PHS = np.repeat([g[0] for g in _GROUPS], C)
_PWS = np.repeat([g[1] for g in _GROUPS], C)
_CS = np.tile(np.arange(C), NKAP // C)

# ---------------------------------------------------------------- host prep

def _build_xb(x):
    """Kappa-major windowed image: xb2[kappa, hq, wq] (class-chunk order,
    padded to 14*128 rows so every front-end chunk is one affine DMA)."""
    xpad = np.zeros((C, 260, 260), np.float32)
    xpad[:, 1:257, 1:257] = x[0]
    xb = np.ascontiguousarray(
        xpad.reshape(C, 65, 4, 65, 4).transpose(2, 4, 0, 1, 3))
    xb2 = np.zeros((128 * NCHUNK14, 65, 65), np.float32)
    xb2[:NKAP] = xb[_PHS % 4, _PWS % 4, _CS]
    return xb2.astype(BF16)


def _build_w4(h, w_qkv, w_dw):
    """Fused (1x1 conv + dw3x3) weights in the kappa basis: [1792, 288].

    Output rows within a head matrix are ordered (khB, cl, kh1, kw) so the
    two fold halves (kh<2, kh>=2) are partition-contiguous."""
    kh = np.arange(4)
    dy = _PHS[:, None] - kh[None, :]            # [1728, 4]
    dx = _PWS[:, None] - kh[None, :]
    my = (dy >= 0) & (dy < 3)
    mx = (dx >= 0) & (dx < 3)
    dyc = np.clip(dy, 0, 2)
    dxc = np.clip(dx, 0, 2)
    w4 = np.zeros((NKAP, 3, CH, 4, 4), np.float32)
    for sel in range(3):
        for cl in range(CH):
            o = sel * C + CH * h + cl
            wd = w_dw[o, 0]
            taps = (wd[dyc[:, :, None], dxc[:, None, :]]
                    * my[:, :, None] * mx[:, None, :])
            w4[:, sel, cl] = w_qkv[o, _CS][:, None, None] * taps
    w4 = (w4.reshape(NKAP, 3, CH, 2, 2, 4).transpose(0, 1, 3, 2, 4, 5)
          .reshape(NKAP, 288))
    w4p = np.zeros((128 * NCHUNK14, 288), np.float32)
    w4p[:NKAP] = w4
    return w4p.astype(BF16)


# ---------------------------------------------------------------- program

_PROG = None

def _build_program():
    import antenv  # noqa: F401
    if "antenv.axon_hooks" not in sys.modules:
        holder = {}
        m = types.ModuleType("antenv.axon_hooks")
        m.set_axon_ntff_profile_hook = lambda hk: holder.__setitem__("h", hk)
        m.get_axon_ntff_profile_hook = lambda: holder.get("h")
        sys.modules["antenv.axon_hooks"] = m
        antenv.axon_hooks = m
        try:
            from trn_agent_boot.trn_boot import _ntff_profile_via_ctypes
            m.set_axon_ntff_profile_hook(
                _ntff_profile_via_ctypes("/opt/axon/libaxon_pjrt.so"))
        except Exception:
            pass

    import concourse.bass as bass
    import concourse.tile as tile
    import concourse.mybir as mybir
    from contextlib import ExitStack

    F32 = mybir.dt.float32
    B16 = mybir.dt.bfloat16
    AF = mybir.ActivationFunctionType

    nc = bass.Bass("TRN2", num_devices=NCORES)

    xb_h = nc.dram_tensor("xb", [128 * NCHUNK14, 65, 65], B16,
                          kind="ExternalInput")
    w4_h = nc.dram_tensor("w4", [128 * NCHUNK14, 288], B16,
                          kind="ExternalInput")
    vcol_h = nc.dram_tensor("vcol", [M96, 2], B16, kind="ExternalInput")
    wpt_h = nc.dram_tensor("wpt", [C, C], B16, kind="ExternalInput")
    id96_h = nc.dram_tensor("id96", [M96, M96], F32, kind="ExternalInput")
    ones_h = nc.dram_tensor("onesrow", [1, M96], F32, kind="ExternalInput")
    onesb_h = nc.dram_tensor("onesb", [1, M96], B16, kind="ExternalInput")
    y_h = nc.dram_tensor("y", [C, 8192], F32, kind="ExternalOutput")
    cc_inA = nc.dram_tensor("cc_inA", [C, 4096], B16)
    cc_outA = nc.dram_tensor("cc_outA", [C, 4096], B16)
    cc_inB = nc.dram_tensor("cc_inB", [C, 4096], B16)
    cc_outB = nc.dram_tensor("cc_outB", [C, 4096], B16)

    with tile.TileContext(nc) as tc, ExitStack() as ctx, \
            nc.allow_low_precision(reason="bf16 compute, fp32 accumulation"):
        const = ctx.enter_context(tc.tile_pool(name="const", bufs=1))
        w4_sb = const.tile([128, NCHUNK14, 288], B16)
        for k in range(NCHUNK14):
            nc.gpsimd.dma_start(w4_sb[:, k, :],
                                w4_h[128 * k:128 * (k + 1), :])
        vcol_sb = const.tile([M96, 2], B16)
        nc.gpsimd.dma_start(vcol_sb[:], vcol_h[:])
        wpt_sb = const.tile([C, C], B16)
        nc.gpsimd.dma_start(wpt_sb[:], wpt_h[:])
        id96_sb = const.tile([M96, M96], F32)
        nc.gpsimd.dma_start(id96_sb[:], id96_h[:])
        ones_sb = const.tile([1, M96], F32)
        nc.gpsimd.dma_start(ones_sb[:], ones_h[:])
        onesb_sb = const.tile([1, M96], B16)
        nc.gpsimd.dma_start(onesb_sb[:], onesb_h[:])

        persist = ctx.enter_context(tc.tile_pool(name="persist", bufs=1))
        qn = persist.tile([M96, N], B16)
        kn = persist.tile([M96, N], B16)
        vt = persist.tile([128, 32 * M96], B16)
        zacc = persist.tile([128, 128], F32)
        rqt = persist.tile([128, 32], F32)

        # ---------------- front end: Q/K/V + column sumsq ----------------
        ctx2 = tc.tile_pool(name="fe_persist", bufs=1)
        fep = ctx2.__enter__()
        vn = fep.tile([M96, N], F32)
        rq_row = fep.tile([1, N], F32)
        rk_row = fep.tile([1, N], B16)
        ph_ps_cm = tc.tile_pool(name="ph1_ps", bufs=1, space="PSUM")
        ph1ps = ph_ps_cm.__enter__()
        rqps = ph1ps.tile([128, 32], F32, bufs=1)
        with tc.tile_pool(name="fe_xp", bufs=3) as xp_pool, \
             tc.tile_pool(name="fe_tmp", bufs=2) as fe_tmp:
            for p in range(NPIECE):
                r0 = 8 * p
                xp_t = xp_pool.tile([128, NCHUNK14, 9, 65], B16,
                                    name="xp", tag="xp")
                for k2 in range(NCHUNK14 // 2):
                    nc.sync.dma_start(
                        xp_t[:, 2 * k2:2 * (k2 + 1), :, :],
                        xb_h[256 * k2:256 * (k2 + 1), r0:r0 + 9, :]
                        .rearrange("(a p) r w -> p a r w", p=128))
                cols = slice(512 * p, 512 * (p + 1))
                for sel, dst in ((0, qn), (1, kn), (2, vn)):
                    pss = ph1ps.tile([M96, 512], F32, name="pss",
                                     tag="ps", bufs=3)
                    for k, (dh, dw) in enumerate(CHUNKS14):
                        nc.tensor.matmul(
                            pss[:],
                            lhsT=w4_sb[:, k, M96 * sel:M96 * (sel + 1)],
                            rhs=xp_t[:, k, dh:dh + 8, dw:dw + 64],
                            start=(k == 0), stop=(k == NCHUNK14 - 1))
                    nc.vector.tensor_copy(dst[:, cols], pss[:])
                    if sel < 2:
                        sq = fe_tmp.tile([M96, 512], B16, name="sq", tag="sq")
                        nc.scalar.activation(sq[:], pss[:], AF.Square)
                        ssp = ph1ps.tile([1, 512], F32, name="ssp",
                                         tag="ssp", bufs=2)
                        nc.tensor.matmul(
                            ssp[:], lhsT=vcol_sb[:, sel:sel + 1], rhs=sq[:],
                            start=True, stop=True)
                        if sel == 0:
                            nc.vector.tensor_copy(rq_row[0:1, cols], ssp[:])
                            for i in range(4):
                                t = 4 * p + i
                                nc.tensor.transpose(
                                    rqps[:, t:t + 1],
                                    rq_row[0:1, 128 * t:128 * (t + 1)],
                                    ones_sb[0:1, 0:1])
                        else:
                            nc.vector.tensor_copy(rk_row[0:1, cols], ssp[:])
                # V^T for this piece's 4 column tiles (keeps PE dense here
                # instead of a transpose-only phase that lets HAM throttle)
                for i in range(4):
                    t = 4 * p + i
                    tp = ph1ps.tile([128, M96], F32, name="tp", tag="tp",
                                    bufs=1)
                    nc.tensor.transpose(
                        tp[:], vn[:, 128 * t:128 * (t + 1)], id96_sb[:])
                    nc.vector.tensor_copy(vt[:, M96 * t:M96 * (t + 1)],
                                          tp[:])

        # ---------------- rqt = rsqrt(sumsq_q) ----------------
        if True:
            nc.vector.reciprocal(rqt[:], rqps[:])
            nc.scalar.activation(rqt[:], rqt[:], AF.Sqrt)

        # ---------------- normalize K ----------------
        with tc.tile_pool(name="nrm", bufs=2) as npool:
            for mt in range(8):
                cols = slice(512 * mt, 512 * (mt + 1))
                bp = ph1ps.tile([M96, 512], F32, name="bp", tag="bp", bufs=1)
                nc.tensor.matmul(bp[:], lhsT=onesb_sb[:],
                                 rhs=rk_row[0:1, cols],
                                 start=True, stop=True)
                b = npool.tile([M96, 512], F32, name="b", tag="b")
                nc.vector.reciprocal(b[:], bp[:])
                brt = npool.tile([M96, 512], B16, name="brt", tag="brt")
                nc.scalar.activation(brt[:], b[:], AF.Sqrt)
                nc.vector.tensor_mul(kn[:, cols], kn[:, cols], brt[:])

        ph_ps_cm.__exit__(None, None, None)
        ctx2.__exit__(None, None, None)
        late = ctx.enter_context(tc.tile_pool(name="late", bufs=1))
        out_acc = late.tile([M96, N], F32)
        out_acc_r = late.tile([M96, N], B16)

        # ---------------- attention (software-pipelined) ----------------
        with tc.tile_pool(name="a_ps", bufs=3, space="PSUM") as apsum, \
             tc.tile_pool(name="o_ps", bufs=2, space="PSUM") as opsum, \
             tc.tile_pool(name="e_sb", bufs=10) as epool, \
             tc.tile_pool(name="z_sb", bufs=2) as zpool, \
             tc.tile_pool(name="vts", bufs=8) as vtspool:

            # per-strip psum spans: 1536+1536+1024 (fewer, wider exp instrs)
            SPANS = [(0, 1536), (1536, 1536), (3072, 1024)]

            def qk_strip(g, tl):
                t = 4 * g + tl
                es = epool.tile([128, N], B16, name="es", tag="es")
                for mp, (c0, w) in enumerate(SPANS):
                    pa = apsum.tile([128, 1536], F32, name="pa", tag="pa",
                                    bufs=2)
                    for half in range(w // 512):
                        nc.tensor.matmul(
                            pa[:, 512 * half:512 * (half + 1)],
                            lhsT=qn[:, 128 * t:128 * (t + 1)],
                            rhs=kn[:, c0 + 512 * half:c0 + 512 * (half + 1)],
                            start=True, stop=True)
                    col = 3 * t + mp
                    nc.scalar.activation(
                        es[:, c0:c0 + w], pa[:, 0:w], AF.Exp,
                        scale=rqt[:, t:t + 1],
                        accum_out=zacc[:, col:col + 1])
                return es

            strips = [qk_strip(0, tl) for tl in range(4)]
            for g in range(8):
                # Z for the group's 4 row-tiles: sum 4 accum cols, +260, 1/x
                zinv = zpool.tile([128, 4], F32)
                nc.vector.tensor_reduce(
                    zinv[:],
                    zacc[:, 12 * g:12 * (g + 1)].rearrange(
                        "p (t m) -> p t m", t=4),
                    axis=mybir.AxisListType.X, op=mybir.AluOpType.add)
                nc.vector.tensor_scalar_add(zinv[:], zinv[:], ZCORR)
                nc.vector.reciprocal(zinv[:], zinv[:])
                vts_tiles = []
                for tl in range(4):
                    t = 4 * g + tl
                    vts = vtspool.tile([128, M96], B16)
                    nc.vector.tensor_scalar_mul(
                        vts[:], vt[:, M96 * t:M96 * (t + 1)],
                        zinv[:, tl:tl + 1])
                    vts_tiles.append(vts)
                nxt = []
                for tl in range(4):
                    if g < 7:
                        nxt.append(qk_strip(g + 1, tl))
                    pos = [opsum.tile([M96, 512], F32, tag="pos",
                                      name=f"pos{jj}")
                           for jj in range(2)]
                    for sl in range(4):
                        for jj in range(2):
                            j = 2 * tl + jj
                            nc.tensor.matmul(
                                pos[jj][:], lhsT=vts_tiles[sl],
                                rhs=strips[sl][:, 512 * j:512 * (j + 1)],
                                start=(sl == 0), stop=(sl == 3))
                    for jj in range(2):
                        j = 2 * tl + jj
                        cols = slice(512 * j, 512 * (j + 1))
                        if g == 0:
                            nc.vector.tensor_copy(
                                out_acc[:, cols], pos[jj][:])
                        else:
                            nc.vector.tensor_add(
                                out_acc[:, cols], out_acc[:, cols],
                                pos[jj][:])
                        if g == 7:
                            # stripe j is final: stage + ship both halves
                            nc.vector.tensor_copy(
                                out_acc_r[:, cols], out_acc[:, cols])
                            nc.sync.dma_start(
                                cc_inA[CH * j:CH * (j + 1), :].rearrange(
                                    "cl (khw i w) -> (cl khw) i w",
                                    khw=8, i=8),
                                out_acc_r[0:48, cols].rearrange(
                                    "p (i w) -> p i w", i=8))
                            nc.sync.dma_start(
                                cc_inB[CH * j:CH * (j + 1), :].rearrange(
                                    "cl (khw i w) -> (cl khw) i w",
                                    khw=8, i=8),
                                out_acc_r[48:96, cols].rearrange(
                                    "p (i w) -> p i w", i=8))
                strips = nxt

        # ---------------- split AllToAll + projection ----------------
        nc.gpsimd.collective_compute(
            "AllToAll", mybir.AluOpType.bypass,
            replica_groups=[list(range(NCORES))],
            ins=[cc_inA[:]], outs=[cc_outA[:]])
        nc.gpsimd.collective_compute(
            "AllToAll", mybir.AluOpType.bypass,
            replica_groups=[list(range(NCORES))],
            ins=[cc_inB[:]], outs=[cc_outB[:]])
        with tc.tile_pool(name="prj", bufs=4) as prj, \
             tc.tile_pool(name="prj_ps", bufs=3, space="PSUM") as prjps, \
             tc.tile_pool(name="yt", bufs=3) as ypool:
            for q in range(16):
                src = cc_outA if q < 8 else cc_outB
                lcols = slice(512 * (q % 8), 512 * (q % 8 + 1))
                fold_t = prj.tile([C, 512], B16)
                nc.sync.dma_start(fold_t[:], src[:, lcols])
                pp = prjps.tile([C, 512], F32)
                nc.tensor.matmul(pp[:], lhsT=wpt_sb[:], rhs=fold_t[:],
                                 start=True, stop=True)
                yt = ypool.tile([C, 512], F32)
                nc.vector.tensor_copy(yt[:], pp[:])
                nc.scalar.dma_start(y_h[:, 512 * q:512 * (q + 1)], yt[:])

    _split_excess_waits(nc)
    return nc


_wsplit_ctr = [0]

def _split_excess_waits(nc, max_waits=1):
    """This walrus build encodes only one sync-wait per instruction; hoist
    extras onto same-engine nops inserted directly before the instruction."""
    import bass_rust
    import concourse.mybir as mybir
    for fn in nc.m.functions:
        for bb in fn.blocks:
            insts = bb.instructions
            out = []
            changed = False
            for inst in insts:
                si = inst.sync_info
                if si is not None and len(si.on_wait) > max_waits:
                    waits = list(si.on_wait)
                    for w in waits[:-max_waits]:
                        _wsplit_ctr[0] += 1
                        nop = bass_rust.InstNoOp(
                            name=f"I-wsplit-{_wsplit_ctr[0]}", ins=[], outs=[])
                        nop.engine = inst.engine
                        nop.sync_info = mybir.SyncInfo(
                            on_wait=[w], on_update=[])
                        out.append(nop)
                    inst.sync_info = mybir.SyncInfo(
                        on_wait=waits[-max_waits:],
                        on_update=list(si.on_update))
                    changed = True
                out.append(inst)
            if changed:
                bb.instructions = out


def _get_program():
    global _PROG
    if _PROG is None:
        _PROG = _build_program()
    return _PROG


# ---------------------------------------------------------------- entry

def kernel(x, w_qkv, w_dw, temperature, w_proj, _trace=False):
    x = np.asarray(x, np.float32)
    w_qkv = np.asarray(w_qkv, np.float32)
    w_dw = np.asarray(w_dw, np.float32)
    temperature = np.asarray(temperature, np.float32)
    w_proj = np.asarray(w_proj, np.float32)

    nc = _get_program()
    from concourse.bass_utils import run_bass_kernel_spmd

    xb = _build_xb(x)
    id96 = np.eye(M96, dtype=np.float32)
    wpt = np.ascontiguousarray(w_proj.T).astype(BF16)
    in_maps = []
    for h in range(NH):
        t_h = float(temperature[h, 0, 0])
        vcol = np.empty((M96, 2), np.float32)
        vcol[:, 0] = 1.0 / (t_h * t_h)
        vcol[:, 1] = 1.0
        in_maps.append({
            "xb": xb,
            "w4": _build_w4(h, w_qkv, w_dw),
            "vcol": vcol.astype(BF16),
            "wpt": wpt,
            "id96": id96,
            "onesrow": np.ones((1, M96), np.float32),
            "onesb": np.ones((1, M96), BF16),
        })

    res = run_bass_kernel_spmd(nc, in_maps, list(range(NCORES)), trace=_trace)

    y = np.empty((1, C, 256, 256), np.float32)
    for s in range(NCORES):
        blk = res.results[s]["y"].reshape(C, 4, 4, 8, GN)
        y[0, :, 32 * s:32 * (s + 1), :] = (
            blk.transpose(0, 3, 1, 4, 2).reshape(C, 32, 256))
    if _trace:
        return y, res
    return y


# revision 25
# speedup vs baseline: 1.0965x; 1.0965x over previous
"""Trainium2 Bass kernel for nn_AttentionV4 (patch attention, 8 heads on 8 cores).

Pipeline per core (= per head h), bf16 compute / fp32 accumulation:
  - The 1x1 qkv conv + depthwise 3x3 conv are fused into one dense 3x3 conv,
    expressed as a single matmul over a 6x6-windowed patch basis:
      Q/K/V[r, n] = sum_kappa W4[kappa, r] * Xp[kappa, n],
    kappa = (ph, pw, c) in [6,6,48] (1728, chunked 14 x 128), n = interior
    patch (64x64 grid = 4096; boundary patches of the stride-4 pad-4 unfold
    are exactly zero and are handled analytically). Xp chunks are DMA'd one
    per 128-kappa chunk via class-affine views of xb (each (dh,dw) class is
    an affine (hm, wm, c) box).
  - Head-matrix rows are ordered (khB, cl, kh1, kw) so each half of the fold
    rows (kh<2 vs kh>=2) is partition-contiguous; the output AllToAll splits
    into two halves, the second overlapping the first half's projection.
  - l2-normalize Q (x temperature) and K per column, A = Qn^T Kn in [-1,1],
    so softmax needs no max subtraction: E = exp(A), Z = rowsum(E) + 260
    (260 = number of zero boundary K columns, each contributing exp(0)).
  - out = (V/Z) @ E; attention is software-pipelined: QK strips of group g+1
    interleave with AV j-block pairs of group g so the PE never waits on exp.
"""
import sys
import types

sys.path.insert(0, "/opt/trn_rl_repo")

import numpy as np
import ml_dtypes

BF16 = ml_dtypes.bfloat16

# ---------------------------------------------------------------- constants
C = 48          # image channels
CH = 6          # channels per head
NH = 8          # heads == cores
GN = 64         # interior patch grid
N = GN * GN     # 4096 interior patches
M96 = 96        # rows of a head matrix (6ch * 4 * 4)
NKAP = 1728     # 36 windows * 48 channels
ZCORR = 260.0   # 4356 - 4096 zero K-columns, exp(0) each
NPIECE = 8      # front-end N pieces (8 patch rows, 512 patches each)
NCORES = 8
NCHUNK14 = 14

# group list (ph, pw) in kappa order
_GROUPS = []
for _dh, _dw in [(0, 0), (0, 1), (1, 0), (1, 1)]:
    for _hm in range(4 if _dh == 0 else 2):
        for _wm in range(4 if _dw == 0 else 2):
            _GROUPS.append((_dh * 4 + _hm, _dw * 4 + _wm))

# (dh, dw) class of each 128-kappa chunk (class boundaries at 768/1152/1536;
# chunk 13's pad rows 1728..1792 are genuine zeros, so its class is harmless)
def _chunk_plan14():
    cls_edges = [(0, (0, 0)), (768, (0, 1)), (1152, (1, 0)), (1536, (1, 1))]
    plan = []
    for k in range(NCHUNK14):
        k0 = 128 * k
        dh, dw = [c for e, c in cls_edges if e <= k0][-1]
        plan.append((dh, dw))
    return plan

CHUNKS14 = _chunk_plan14()

# kappa order (ph, pw, c), c fastest within each group
_PHS = np.repeat([g[0] for g in _GROUPS], C)
_PWS = np.repeat([g[1] for g in _GROUPS], C)
_CS = np.tile(np.arange(C), NKAP // C)

# ---------------------------------------------------------------- host prep

def _build_xb(x):
    """Kappa-major windowed image: xb2[kappa, hq, wq] (class-chunk order,
    padded to 14*128 rows so every front-end chunk is one affine DMA)."""
    xpad = np.zeros((C, 260, 260), np.float32)
    xpad[:, 1:257, 1:257] = x[0]
    xb = np.ascontiguousarray(
        xpad.reshape(C, 65, 4, 65, 4).transpose(2, 4, 0, 1, 3))
    xb2 = np.zeros((128 * NCHUNK14, 65, 65), np.float32)
    xb2[:NKAP] = xb[_PHS % 4, _PWS % 4, _CS]
    return xb2.astype(BF16)


def _build_w4(h, w_qkv, w_dw):
    """Fused (1x1 conv + dw3x3) weights in the kappa basis: [1792, 288].

    Output rows within a head matrix are ordered (khB, cl, kh1, kw) so the
    two fold halves (kh<2, kh>=2) are partition-contiguous."""
    kh = np.arange(4)
    dy = _PHS[:, None] - kh[None, :]            # [1728, 4]
    dx = _PWS[:, None] - kh[None, :]
    my = (dy >= 0) & (dy < 3)
    mx = (dx >= 0) & (dx < 3)
    dyc = np.clip(dy, 0, 2)
    dxc = np.clip(dx, 0, 2)
    w4 = np.zeros((NKAP, 3, CH, 4, 4), np.float32)
    for sel in range(3):
        for cl in range(CH):
            o = sel * C + CH * h + cl
            wd = w_dw[o, 0]
            taps = (wd[dyc[:, :, None], dxc[:, None, :]]
                    * my[:, :, None] * mx[:, None, :])
            w4[:, sel, cl] = w_qkv[o, _CS][:, None, None] * taps
    w4 = (w4.reshape(NKAP, 3, CH, 2, 2, 4).transpose(0, 1, 3, 2, 4, 5)
          .reshape(NKAP, 288))
    w4p = np.zeros((128 * NCHUNK14, 288), np.float32)
    w4p[:NKAP] = w4
    return w4p.astype(BF16)


# ---------------------------------------------------------------- program

_PROG = None

def _build_program():
    import antenv  # noqa: F401
    if "antenv.axon_hooks" not in sys.modules:
        holder = {}
        m = types.ModuleType("antenv.axon_hooks")
        m.set_axon_ntff_profile_hook = lambda hk: holder.__setitem__("h", hk)
        m.get_axon_ntff_profile_hook = lambda: holder.get("h")
        sys.modules["antenv.axon_hooks"] = m
        antenv.axon_hooks = m
        try:
            from trn_agent_boot.trn_boot import _ntff_profile_via_ctypes
            m.set_axon_ntff_profile_hook(
                _ntff_profile_via_ctypes("/opt/axon/libaxon_pjrt.so"))
        except Exception:
            pass

    import concourse.bass as bass
    import concourse.tile as tile
    import concourse.mybir as mybir
    from contextlib import ExitStack

    F32 = mybir.dt.float32
    B16 = mybir.dt.bfloat16
    AF = mybir.ActivationFunctionType

    nc = bass.Bass("TRN2", num_devices=NCORES)

    xb_h = nc.dram_tensor("xb", [128 * NCHUNK14, 65, 65], B16,
                          kind="ExternalInput")
    w4_h = nc.dram_tensor("w4", [128 * NCHUNK14, 288], B16,
                          kind="ExternalInput")
    vcol_h = nc.dram_tensor("vcol", [M96, 2], B16, kind="ExternalInput")
    wpt_h = nc.dram_tensor("wpt", [C, C], B16, kind="ExternalInput")
    id96_h = nc.dram_tensor("id96", [M96, M96], F32, kind="ExternalInput")
    ones_h = nc.dram_tensor("onesrow", [1, M96], F32, kind="ExternalInput")
    onesb_h = nc.dram_tensor("onesb", [1, M96], B16, kind="ExternalInput")
    y_h = nc.dram_tensor("y", [C, 8192], F32, kind="ExternalOutput")
    cc_inA = nc.dram_tensor("cc_inA", [C, 4096], B16)
    cc_outA = nc.dram_tensor("cc_outA", [C, 4096], B16)
    cc_inB = nc.dram_tensor("cc_inB", [C, 4096], B16)
    cc_outB = nc.dram_tensor("cc_outB", [C, 4096], B16)

    with tile.TileContext(nc) as tc, ExitStack() as ctx, \
            nc.allow_low_precision(reason="bf16 compute, fp32 accumulation"):
        const = ctx.enter_context(tc.tile_pool(name="const", bufs=1))
        w4_sb = const.tile([128, NCHUNK14, 288], B16)
        for k in range(NCHUNK14):
            nc.gpsimd.dma_start(w4_sb[:, k, :],
                                w4_h[128 * k:128 * (k + 1), :])
        vcol_sb = const.tile([M96, 2], B16)
        nc.gpsimd.dma_start(vcol_sb[:], vcol_h[:])
        wpt_sb = const.tile([C, C], B16)
        nc.gpsimd.dma_start(wpt_sb[:], wpt_h[:])
        id96_sb = const.tile([M96, M96], F32)
        nc.gpsimd.dma_start(id96_sb[:], id96_h[:])
        ones_sb = const.tile([1, M96], F32)
        nc.gpsimd.dma_start(ones_sb[:], ones_h[:])
        onesb_sb = const.tile([1, M96], B16)
        nc.gpsimd.dma_start(onesb_sb[:], onesb_h[:])

        persist = ctx.enter_context(tc.tile_pool(name="persist", bufs=1))
        qn = persist.tile([M96, N], B16)
        kn = persist.tile([M96, N], B16)
        vt = persist.tile([128, 32 * M96], B16)
        zacc = persist.tile([128, 128], F32)
        rqt = persist.tile([128, 32], F32)

        # ---------------- front end: Q/K/V + column sumsq ----------------
        ctx2 = tc.tile_pool(name="fe_persist", bufs=1)
        fep = ctx2.__enter__()
        vn = fep.tile([M96, N], F32)
        rq_row = fep.tile([1, N], F32)
        rk_row = fep.tile([1, N], B16)
        ph_ps_cm = tc.tile_pool(name="ph1_ps", bufs=1, space="PSUM")
        ph1ps = ph_ps_cm.__enter__()
        with tc.tile_pool(name="fe_xp", bufs=3) as xp_pool, \
             tc.tile_pool(name="fe_tmp", bufs=2) as fe_tmp:
            for p in range(NPIECE):
                r0 = 8 * p
                xp_t = xp_pool.tile([128, NCHUNK14, 9, 65], B16,
                                    name="xp", tag="xp")
                for k2 in range(NCHUNK14 // 2):
                    nc.sync.dma_start(
                        xp_t[:, 2 * k2:2 * (k2 + 1), :, :],
                        xb_h[256 * k2:256 * (k2 + 1), r0:r0 + 9, :]
                        .rearrange("(a p) r w -> p a r w", p=128))
                cols = slice(512 * p, 512 * (p + 1))
                for sel, dst in ((0, qn), (1, kn), (2, vn)):
                    pss = ph1ps.tile([M96, 512], F32, name="pss",
                                     tag="ps", bufs=3)
                    for k, (dh, dw) in enumerate(CHUNKS14):
                        nc.tensor.matmul(
                            pss[:],
                            lhsT=w4_sb[:, k, M96 * sel:M96 * (sel + 1)],
                            rhs=xp_t[:, k, dh:dh + 8, dw:dw + 64],
                            start=(k == 0), stop=(k == NCHUNK14 - 1))
                    nc.vector.tensor_copy(dst[:, cols], pss[:])
                    if sel < 2:
                        sq = fe_tmp.tile([M96, 512], B16, name="sq", tag="sq")
                        nc.scalar.activation(sq[:], pss[:], AF.Square)
                        ssp = ph1ps.tile([1, 512], F32, name="ssp",
                                         tag="ssp", bufs=2)
                        nc.tensor.matmul(
                            ssp[:], lhsT=vcol_sb[:, sel:sel + 1], rhs=sq[:],
                            start=True, stop=True)
                        if sel == 0:
                            nc.vector.tensor_copy(rq_row[0:1, cols], ssp[:])
                        else:
                            nc.vector.tensor_copy(rk_row[0:1, cols], ssp[:])

        # ---------------- rqt = rsqrt(sumsq_q) in row-tile layout ----------------
        if True:
            rqps = ph1ps.tile([128, 32], F32, bufs=1)
            for t in range(32):
                nc.tensor.transpose(
                    rqps[:, t:t + 1], rq_row[0:1, 128 * t:128 * (t + 1)],
                    ones_sb[0:1, 0:1])
            nc.vector.reciprocal(rqt[:], rqps[:])
            nc.scalar.activation(rqt[:], rqt[:], AF.Sqrt)

        # ---------------- normalize K ----------------
        with tc.tile_pool(name="nrm", bufs=2) as npool:
            for mt in range(8):
                cols = slice(512 * mt, 512 * (mt + 1))
                bp = ph1ps.tile([M96, 512], F32, name="bp", tag="bp", bufs=1)
                nc.tensor.matmul(bp[:], lhsT=onesb_sb[:],
                                 rhs=rk_row[0:1, cols],
                                 start=True, stop=True)
                b = npool.tile([M96, 512], F32, name="b", tag="b")
                nc.vector.reciprocal(b[:], bp[:])
                brt = npool.tile([M96, 512], B16, name="brt", tag="brt")
                nc.scalar.activation(brt[:], b[:], AF.Sqrt)
                nc.vector.tensor_mul(kn[:, cols], kn[:, cols], brt[:])

        # ---------------- V^T via PE transpose ----------------
        if True:
            for t in range(32):
                tp = ph1ps.tile([128, M96], F32, name="tp", tag="tp", bufs=1)
                nc.tensor.transpose(
                    tp[:], vn[:, 128 * t:128 * (t + 1)], id96_sb[:])
                nc.vector.tensor_copy(vt[:, M96 * t:M96 * (t + 1)], tp[:])
        ph_ps_cm.__exit__(None, None, None)
        ctx2.__exit__(None, None, None)
        late = ctx.enter_context(tc.tile_pool(name="late", bufs=1))
        out_acc = late.tile([M96, N], F32)
        out_acc_r = late.tile([M96, N], B16)

        # ---------------- attention (software-pipelined) ----------------
        with tc.tile_pool(name="a_ps", bufs=3, space="PSUM") as apsum, \
             tc.tile_pool(name="o_ps", bufs=2, space="PSUM") as opsum, \
             tc.tile_pool(name="e_sb", bufs=10) as epool, \
             tc.tile_pool(name="z_sb", bufs=2) as zpool, \
             tc.tile_pool(name="vts", bufs=8) as vtspool:

            def qk_chunk(es, t, mp):
                # one [128, 1024] A-psum chunk + its exp
                pa = apsum.tile([128, 1024], F32, name="pa", tag="pa",
                                bufs=3)
                for half in range(2):
                    nc.tensor.matmul(
                        pa[:, 512 * half:512 * (half + 1)],
                        lhsT=qn[:, 128 * t:128 * (t + 1)],
                        rhs=kn[:, 1024 * mp + 512 * half:
                               1024 * mp + 512 * (half + 1)],
                        start=True, stop=True)
                col = 4 * t + mp
                nc.scalar.activation(
                    es[:, 1024 * mp:1024 * (mp + 1)], pa[:], AF.Exp,
                    scale=rqt[:, t:t + 1],
                    accum_out=zacc[:, col:col + 1])

            def qk_strip(g, tl):
                t = 4 * g + tl
                es = epool.tile([128, N], B16, name="es", tag="es")
                for mp in range(4):
                    qk_chunk(es, t, mp)
                return es

            strips = [qk_strip(0, tl) for tl in range(4)]
            for g in range(8):
                # Z for the group's 4 row-tiles: sum 4 accum cols, +260, 1/x
                zinv = zpool.tile([128, 4], F32)
                nc.vector.tensor_reduce(
                    zinv[:],
                    zacc[:, 16 * g:16 * (g + 1)].rearrange(
                        "p (t m) -> p t m", t=4),
                    axis=mybir.AxisListType.X, op=mybir.AluOpType.add)
                nc.vector.tensor_scalar_add(zinv[:], zinv[:], ZCORR)
                nc.vector.reciprocal(zinv[:], zinv[:])
                vts_tiles = []
                for tl in range(4):
                    t = 4 * g + tl
                    vts = vtspool.tile([128, M96], B16)
                    nc.vector.tensor_scalar_mul(
                        vts[:], vt[:, M96 * t:M96 * (t + 1)],
                        zinv[:, tl:tl + 1])
                    vts_tiles.append(vts)
                nxt = []
                for tl in range(4):
                    # interleave next group's QK chunks with this group's AV
                    # at 2-MM granularity so a psum-starved QK never leaves
                    # the in-order PE without ready AV work right behind it
                    es2 = None
                    if g < 7:
                        es2 = epool.tile([128, N], B16, name="es", tag="es")
                        nxt.append(es2)
                    pos = [opsum.tile([M96, 512], F32, tag="pos",
                                      name=f"pos{jj}")
                           for jj in range(2)]
                    for sl in range(4):
                        for jj in range(2):
                            j = 2 * tl + jj
                            nc.tensor.matmul(
                                pos[jj][:], lhsT=vts_tiles[sl],
                                rhs=strips[sl][:, 512 * j:512 * (j + 1)],
                                start=(sl == 0), stop=(sl == 3))
                        if es2 is not None:
                            qk_chunk(es2, 4 * (g + 1) + tl, sl)
                    for jj in range(2):
                        j = 2 * tl + jj
                        cols = slice(512 * j, 512 * (j + 1))
                        if g == 0:
                            nc.vector.tensor_copy(
                                out_acc[:, cols], pos[jj][:])
                        else:
                            nc.vector.tensor_add(
                                out_acc[:, cols], out_acc[:, cols],
                                pos[jj][:])
                        if g == 7:
                            # stripe j is final: stage + ship both halves
                            nc.vector.tensor_copy(
                                out_acc_r[:, cols], out_acc[:, cols])
                            nc.sync.dma_start(
                                cc_inA[CH * j:CH * (j + 1), :].rearrange(
                                    "cl (khw i w) -> (cl khw) i w",
                                    khw=8, i=8),
                                out_acc_r[0:48, cols].rearrange(
                                    "p (i w) -> p i w", i=8))
                            nc.sync.dma_start(
                                cc_inB[CH * j:CH * (j + 1), :].rearrange(
                                    "cl (khw i w) -> (cl khw) i w",
                                    khw=8, i=8),
                                out_acc_r[48:96, cols].rearrange(
                                    "p (i w) -> p i w", i=8))
                strips = nxt

        # ---------------- split AllToAll + projection ----------------
        nc.gpsimd.collective_compute(
            "AllToAll", mybir.AluOpType.bypass,
            replica_groups=[list(range(NCORES))],
            ins=[cc_inA[:]], outs=[cc_outA[:]])
        nc.gpsimd.collective_compute(
            "AllToAll", mybir.AluOpType.bypass,
            replica_groups=[list(range(NCORES))],
            ins=[cc_inB[:]], outs=[cc_outB[:]])
        with tc.tile_pool(name="prj", bufs=4) as prj, \
             tc.tile_pool(name="prj_ps", bufs=3, space="PSUM") as prjps, \
             tc.tile_pool(name="yt", bufs=3) as ypool:
            for q in range(16):
                src = cc_outA if q < 8 else cc_outB
                lcols = slice(512 * (q % 8), 512 * (q % 8 + 1))
                fold_t = prj.tile([C, 512], B16)
                nc.sync.dma_start(fold_t[:], src[:, lcols])
                pp = prjps.tile([C, 512], F32)
                nc.tensor.matmul(pp[:], lhsT=wpt_sb[:], rhs=fold_t[:],
                                 start=True, stop=True)
                yt = ypool.tile([C, 512], F32)
                nc.vector.tensor_copy(yt[:], pp[:])
                nc.scalar.dma_start(y_h[:, 512 * q:512 * (q + 1)], yt[:])

    _skip_redundant_ldweights(nc)
    _split_excess_waits(nc)
    return nc


_ldwskip_ctr = [0]

def _skip_redundant_ldweights(nc):
    """An InstLdweights whose stationary operand matches the weights already
    sitting in the PE array (loaded by the previous InstLdweights, with only
    non-transpose matmuls in between) is redundant: the array state is
    unchanged. Convert it to a NoOp that keeps its sync_info."""
    import bass_rust
    import concourse.mybir as mybir
    PE = mybir.EngineType.PE

    def wsig(w):
        return (w.memref, w.offset, str(w.ap), str(w.dtype))

    for fn in nc.m.functions:
        for bb in fn.blocks:
            last = None
            out = []
            changed = False
            for inst in bb.instructions:
                if getattr(inst, "engine", None) == PE:
                    tn = type(inst).__name__
                    if tn == "InstLdweights":
                        s = wsig(inst.ins[0])
                        if s == last:
                            _ldwskip_ctr[0] += 1
                            nop = bass_rust.InstNoOp(
                                name=f"I-ldwskip-{_ldwskip_ctr[0]}",
                                ins=[], outs=[])
                            nop.engine = PE
                            nop.sync_info = inst.sync_info
                            out.append(nop)
                            changed = True
                            continue
                        last = s
                    elif tn == "InstMatmult":
                        if inst.is_transpose:
                            last = None
                    elif tn not in ("InstNoOp", "InstEventSemaphore"):
                        last = None
                out.append(inst)
            if changed:
                bb.instructions = out


_wsplit_ctr = [0]

def _split_excess_waits(nc, max_waits=1):
    """This walrus build encodes only one sync-wait per instruction; hoist
    extras onto same-engine nops inserted directly before the instruction."""
    import bass_rust
    import concourse.mybir as mybir
    for fn in nc.m.functions:
        for bb in fn.blocks:
            insts = bb.instructions
            out = []
            changed = False
            for inst in insts:
                si = inst.sync_info
                if si is not None and len(si.on_wait) > max_waits:
                    waits = list(si.on_wait)
                    for w in waits[:-max_waits]:
                        _wsplit_ctr[0] += 1
                        nop = bass_rust.InstNoOp(
                            name=f"I-wsplit-{_wsplit_ctr[0]}", ins=[], outs=[])
                        nop.engine = inst.engine
                        nop.sync_info = mybir.SyncInfo(
                            on_wait=[w], on_update=[])
                        out.append(nop)
                    inst.sync_info = mybir.SyncInfo(
                        on_wait=waits[-max_waits:],
                        on_update=list(si.on_update))
                    changed = True
                out.append(inst)
            if changed:
                bb.instructions = out


def _get_program():
    global _PROG
    if _PROG is None:
        _PROG = _build_program()
    return _PROG


# ---------------------------------------------------------------- entry

def kernel(x, w_qkv, w_dw, temperature, w_proj, _trace=False):
    x = np.asarray(x, np.float32)
    w_qkv = np.asarray(w_qkv, np.float32)
    w_dw = np.asarray(w_dw, np.float32)
    temperature = np.asarray(temperature, np.float32)
    w_proj = np.asarray(w_proj, np.float32)

    nc = _get_program()
    from concourse.bass_utils import run_bass_kernel_spmd

    xb = _build_xb(x)
    id96 = np.eye(M96, dtype=np.float32)
    wpt = np.ascontiguousarray(w_proj.T).astype(BF16)
    in_maps = []
    for h in range(NH):
        t_h = float(temperature[h, 0, 0])
        vcol = np.empty((M96, 2), np.float32)
        vcol[:, 0] = 1.0 / (t_h * t_h)
        vcol[:, 1] = 1.0
        in_maps.append({
            "xb": xb,
            "w4": _build_w4(h, w_qkv, w_dw),
            "vcol": vcol.astype(BF16),
            "wpt": wpt,
            "id96": id96,
            "onesrow": np.ones((1, M96), np.float32),
            "onesb": np.ones((1, M96), BF16),
        })

    res = run_bass_kernel_spmd(nc, in_maps, list(range(NCORES)), trace=_trace)

    y = np.empty((1, C, 256, 256), np.float32)
    for s in range(NCORES):
        blk = res.results[s]["y"].reshape(C, 4, 4, 8, GN)
        y[0, :, 32 * s:32 * (s + 1), :] = (
            blk.transpose(0, 3, 1, 4, 2).reshape(C, 32, 256))
    if _trace:
        return y, res
    return y


# revision 31
# speedup vs baseline: 1.1602x; 1.0581x over previous
"""Trainium2 Bass kernel for nn_AttentionV4 (patch attention, 8 heads on 8 cores).

Pipeline per core (= per head h), bf16 compute / fp32 accumulation:
  - The 1x1 qkv conv + depthwise 3x3 conv are fused into one dense 3x3 conv,
    expressed as a single matmul over a 6x6-windowed patch basis:
      Q/K/V[r, n] = sum_kappa W4[kappa, r] * Xp[kappa, n],
    kappa = (ph, pw, c) in [6,6,48] (1728, chunked 14 x 128), n = interior
    patch (64x64 grid = 4096; boundary patches of the stride-4 pad-4 unfold
    are exactly zero and are handled analytically). Xp chunks are DMA'd one
    per 128-kappa chunk via class-affine views of xb (each (dh,dw) class is
    an affine (hm, wm, c) box).
  - Head-matrix rows are ordered (khB, cl, kh1, kw) so each half of the fold
    rows (kh<2 vs kh>=2) is partition-contiguous; the output AllToAll splits
    into two halves, the second overlapping the first half's projection.
  - l2-normalize Q (x temperature) and K per column, A = Qn^T Kn in [-1,1],
    so softmax needs no max subtraction: E = exp(A), Z = rowsum(E) + 260
    (260 = number of zero boundary K columns, each contributing exp(0)).
  - out = (V/Z) @ E; attention is software-pipelined: QK strips of group g+1
    interleave with AV j-block pairs of group g so the PE never waits on exp.
"""
import sys
import types

sys.path.insert(0, "/opt/trn_rl_repo")

import numpy as np
import ml_dtypes

BF16 = ml_dtypes.bfloat16

# ---------------------------------------------------------------- constants
C = 48          # image channels
CH = 6          # channels per head
NH = 8          # heads == cores
GN = 64         # interior patch grid
N = GN * GN     # 4096 interior patches
M96 = 96        # rows of a head matrix (6ch * 4 * 4)
NKAP = 1728     # 36 windows * 48 channels
ZCORR = 260.0   # 4356 - 4096 zero K-columns, exp(0) each
NPIECE = 8      # front-end N pieces (8 patch rows, 512 patches each)
NCORES = 8
NCHUNK14 = 14

# group list (ph, pw) in kappa order
_GROUPS = []
for _dh, _dw in [(0, 0), (0, 1), (1, 0), (1, 1)]:
    for _hm in range(4 if _dh == 0 else 2):
        for _wm in range(4 if _dw == 0 else 2):
            _GROUPS.append((_dh * 4 + _hm, _dw * 4 + _wm))

# (dh, dw) class of each 128-kappa chunk (class boundaries at 768/1152/1536;
# chunk 13's pad rows 1728..1792 are genuine zeros, so its class is harmless)
def _chunk_plan14():
    cls_edges = [(0, (0, 0)), (768, (0, 1)), (1152, (1, 0)), (1536, (1, 1))]
    plan = []
    for k in range(NCHUNK14):
        k0 = 128 * k
        dh, dw = [c for e, c in cls_edges if e <= k0][-1]
        plan.append((dh, dw))
    return plan

CHUNKS14 = _chunk_plan14()

# kappa order (ph, pw, c), c fastest within each group
_PHS = np.repeat([g[0] for g in _GROUPS], C)
_PWS = np.repeat([g[1] for g in _GROUPS], C)
_CS = np.tile(np.arange(C), NKAP // C)

# ---------------------------------------------------------------- host prep

def _build_xb(x):
    """Kappa-major windowed image: xb2[kappa, hq, wq] (class-chunk order,
    padded to 14*128 rows so every front-end chunk is one affine DMA)."""
    xpad = np.zeros((C, 260, 260), np.float32)
    xpad[:, 1:257, 1:257] = x[0]
    xb = np.ascontiguousarray(
        xpad.reshape(C, 65, 4, 65, 4).transpose(2, 4, 0, 1, 3))
    xb2 = np.zeros((128 * NCHUNK14, 65, 65), np.float32)
    xb2[:NKAP] = xb[_PHS % 4, _PWS % 4, _CS]
    return xb2.astype(BF16)


def _build_w4(h, w_qkv, w_dw):
    """Fused (1x1 conv + dw3x3) weights in the kappa basis: [1792, 288].

    Output rows within a head matrix are ordered (khB, cl, kh1, kw) so the
    two fold halves (kh<2, kh>=2) are partition-contiguous."""
    kh = np.arange(4)
    dy = _PHS[:, None] - kh[None, :]            # [1728, 4]
    dx = _PWS[:, None] - kh[None, :]
    my = (dy >= 0) & (dy < 3)
    mx = (dx >= 0) & (dx < 3)
    dyc = np.clip(dy, 0, 2)
    dxc = np.clip(dx, 0, 2)
    w4 = np.zeros((NKAP, 3, CH, 4, 4), np.float32)
    for sel in range(3):
        for cl in range(CH):
            o = sel * C + CH * h + cl
            wd = w_dw[o, 0]
            taps = (wd[dyc[:, :, None], dxc[:, None, :]]
                    * my[:, :, None] * mx[:, None, :])
            w4[:, sel, cl] = w_qkv[o, _CS][:, None, None] * taps
    w4 = (w4.reshape(NKAP, 3, CH, 2, 2, 4).transpose(0, 1, 3, 2, 4, 5)
          .reshape(NKAP, 288))
    w4p = np.zeros((128 * NCHUNK14, 288), np.float32)
    w4p[:NKAP] = w4
    return w4p.astype(BF16)


# ---------------------------------------------------------------- program

_PROG = None

def _build_program():
    import antenv  # noqa: F401
    if "antenv.axon_hooks" not in sys.modules:
        holder = {}
        m = types.ModuleType("antenv.axon_hooks")
        m.set_axon_ntff_profile_hook = lambda hk: holder.__setitem__("h", hk)
        m.get_axon_ntff_profile_hook = lambda: holder.get("h")
        sys.modules["antenv.axon_hooks"] = m
        antenv.axon_hooks = m
        try:
            from trn_agent_boot.trn_boot import _ntff_profile_via_ctypes
            m.set_axon_ntff_profile_hook(
                _ntff_profile_via_ctypes("/opt/axon/libaxon_pjrt.so"))
        except Exception:
            pass

    import concourse.bass as bass
    import concourse.tile as tile
    import concourse.mybir as mybir
    from contextlib import ExitStack

    F32 = mybir.dt.float32
    B16 = mybir.dt.bfloat16
    AF = mybir.ActivationFunctionType

    nc = bass.Bass("TRN2", num_devices=NCORES)

    xb_h = nc.dram_tensor("xb", [128 * NCHUNK14, 65, 65], B16,
                          kind="ExternalInput")
    w4_h = nc.dram_tensor("w4", [128 * NCHUNK14, 288], B16,
                          kind="ExternalInput")
    vcol_h = nc.dram_tensor("vcol", [M96, 2], B16, kind="ExternalInput")
    wpt_h = nc.dram_tensor("wpt", [C, C], B16, kind="ExternalInput")
    id96_h = nc.dram_tensor("id96", [M96, M96], F32, kind="ExternalInput")
    ones_h = nc.dram_tensor("onesrow", [1, M96], F32, kind="ExternalInput")
    onesb_h = nc.dram_tensor("onesb", [1, M96], B16, kind="ExternalInput")
    y_h = nc.dram_tensor("y", [C, 8192], F32, kind="ExternalOutput")
    cc_inA = nc.dram_tensor("cc_inA", [C, 4096], B16)
    cc_outA = nc.dram_tensor("cc_outA", [C, 4096], B16)
    cc_inB = nc.dram_tensor("cc_inB", [C, 4096], B16)
    cc_outB = nc.dram_tensor("cc_outB", [C, 4096], B16)

    with tile.TileContext(nc) as tc, ExitStack() as ctx, \
            nc.allow_low_precision(reason="bf16 compute, fp32 accumulation"):
        const = ctx.enter_context(tc.tile_pool(name="const", bufs=1))
        w4_sb = const.tile([128, NCHUNK14, 288], B16)
        for k in range(NCHUNK14):
            nc.gpsimd.dma_start(w4_sb[:, k, :],
                                w4_h[128 * k:128 * (k + 1), :])
        vcol_sb = const.tile([M96, 2], B16)
        nc.gpsimd.dma_start(vcol_sb[:], vcol_h[:])
        wpt_sb = const.tile([C, C], B16)
        nc.gpsimd.dma_start(wpt_sb[:], wpt_h[:])
        id96_sb = const.tile([M96, M96], F32)
        nc.gpsimd.dma_start(id96_sb[:], id96_h[:])
        ones_sb = const.tile([1, M96], F32)
        nc.gpsimd.dma_start(ones_sb[:], ones_h[:])
        onesb_sb = const.tile([1, M96], B16)
        nc.gpsimd.dma_start(onesb_sb[:], onesb_h[:])

        persist = ctx.enter_context(tc.tile_pool(name="persist", bufs=1))
        qn = persist.tile([M96, N], B16)
        kn = persist.tile([M96, N], B16)
        vt = persist.tile([128, 32 * M96], B16)
        zacc = persist.tile([128, 128], F32)
        rqt = persist.tile([128, 32], F32)

        # ---------------- front end: Q/K/V + column sumsq ----------------
        ctx2 = tc.tile_pool(name="fe_persist", bufs=1)
        fep = ctx2.__enter__()
        vn = fep.tile([M96, N], F32)
        rq_row = fep.tile([1, N], F32)
        rk_row = fep.tile([1, N], B16)
        ph_ps_cm = tc.tile_pool(name="ph1_ps", bufs=1, space="PSUM")
        ph1ps = ph_ps_cm.__enter__()
        rqps = ph1ps.tile([128, 32], F32, bufs=1)
        with tc.tile_pool(name="fe_xp", bufs=3) as xp_pool, \
             tc.tile_pool(name="fe_tmp", bufs=2) as fe_tmp:
            for p in range(NPIECE):
                r0 = 8 * p
                xp_t = xp_pool.tile([128, NCHUNK14, 9, 65], B16,
                                    name="xp", tag="xp")
                for k2 in range(NCHUNK14 // 2):
                    nc.sync.dma_start(
                        xp_t[:, 2 * k2:2 * (k2 + 1), :, :],
                        xb_h[256 * k2:256 * (k2 + 1), r0:r0 + 9, :]
                        .rearrange("(a p) r w -> p a r w", p=128))
                cols = slice(512 * p, 512 * (p + 1))
                for sel, dst in ((0, qn), (1, kn), (2, vn)):
                    pss = ph1ps.tile([M96, 512], F32, name="pss",
                                     tag="ps", bufs=3)
                    for k, (dh, dw) in enumerate(CHUNKS14):
                        nc.tensor.matmul(
                            pss[:],
                            lhsT=w4_sb[:, k, M96 * sel:M96 * (sel + 1)],
                            rhs=xp_t[:, k, dh:dh + 8, dw:dw + 64],
                            start=(k == 0), stop=(k == NCHUNK14 - 1))
                    nc.vector.tensor_copy(dst[:, cols], pss[:])
                    if sel < 2:
                        sq = fe_tmp.tile([M96, 512], B16, name="sq", tag="sq")
                        nc.scalar.activation(sq[:], pss[:], AF.Square)
                        ssp = ph1ps.tile([1, 512], F32, name="ssp",
                                         tag="ssp", bufs=2)
                        nc.tensor.matmul(
                            ssp[:], lhsT=vcol_sb[:, sel:sel + 1], rhs=sq[:],
                            start=True, stop=True)
                        if sel == 0:
                            nc.vector.tensor_copy(rq_row[0:1, cols], ssp[:])
                            for i in range(4):
                                t = 4 * p + i
                                nc.tensor.transpose(
                                    rqps[:, t:t + 1],
                                    rq_row[0:1, 128 * t:128 * (t + 1)],
                                    ones_sb[0:1, 0:1])
                        else:
                            nc.vector.tensor_copy(rk_row[0:1, cols], ssp[:])
                            # normalize this piece's K columns immediately
                            bp = ph1ps.tile([M96, 512], F32, name="bp",
                                            tag="bp", bufs=1)
                            nc.tensor.matmul(bp[:], lhsT=onesb_sb[:],
                                             rhs=rk_row[0:1, cols],
                                             start=True, stop=True)
                            b = fe_tmp.tile([M96, 512], F32, name="b",
                                            tag="b")
                            nc.vector.reciprocal(b[:], bp[:])
                            brt = fe_tmp.tile([M96, 512], B16, name="brt",
                                              tag="brt")
                            nc.scalar.activation(brt[:], b[:], AF.Sqrt)
                            nc.vector.tensor_mul(kn[:, cols], kn[:, cols],
                                                 brt[:])

        # ---------------- rqt = rsqrt(sumsq_q) ----------------
        if True:
            nc.vector.reciprocal(rqt[:], rqps[:])
            nc.scalar.activation(rqt[:], rqt[:], AF.Sqrt)

        # ---------------- V^T via PE transpose ----------------
        if True:
            for t in range(32):
                tp = ph1ps.tile([128, M96], F32, name="tp", tag="tp", bufs=1)
                nc.tensor.transpose(
                    tp[:], vn[:, 128 * t:128 * (t + 1)], id96_sb[:])
                nc.vector.tensor_copy(vt[:, M96 * t:M96 * (t + 1)], tp[:])
        ph_ps_cm.__exit__(None, None, None)
        ctx2.__exit__(None, None, None)
        late = ctx.enter_context(tc.tile_pool(name="late", bufs=1))
        out_acc = late.tile([M96, N], F32)
        out_acc_r = late.tile([M96, N], B16)

        # ---------------- attention (software-pipelined) ----------------
        with tc.tile_pool(name="a_ps", bufs=3, space="PSUM") as apsum, \
             tc.tile_pool(name="o_ps", bufs=2, space="PSUM") as opsum, \
             tc.tile_pool(name="e_sb", bufs=10) as epool, \
             tc.tile_pool(name="z_sb", bufs=2) as zpool, \
             tc.tile_pool(name="vts", bufs=8) as vtspool:

            def qk_chunk(es, t, mp):
                # one [128, 1024] A-psum chunk + its exp
                pa = apsum.tile([128, 1024], F32, name="pa", tag="pa",
                                bufs=3)
                for half in range(2):
                    nc.tensor.matmul(
                        pa[:, 512 * half:512 * (half + 1)],
                        lhsT=qn[:, 128 * t:128 * (t + 1)],
                        rhs=kn[:, 1024 * mp + 512 * half:
                               1024 * mp + 512 * (half + 1)],
                        start=True, stop=True)
                col = 4 * t + mp
                nc.scalar.activation(
                    es[:, 1024 * mp:1024 * (mp + 1)], pa[:], AF.Exp,
                    scale=rqt[:, t:t + 1],
                    accum_out=zacc[:, col:col + 1])

            def qk_strip(g, tl):
                t = 4 * g + tl
                es = epool.tile([128, N], B16, name="es", tag="es")
                for mp in range(4):
                    qk_chunk(es, t, mp)
                return es

            strips = [qk_strip(0, tl) for tl in range(4)]
            for g in range(8):
                # Z for the group's 4 row-tiles: sum 4 accum cols, +260, 1/x
                zinv = zpool.tile([128, 4], F32)
                nc.vector.tensor_reduce(
                    zinv[:],
                    zacc[:, 16 * g:16 * (g + 1)].rearrange(
                        "p (t m) -> p t m", t=4),
                    axis=mybir.AxisListType.X, op=mybir.AluOpType.add)
                nc.vector.tensor_scalar_add(zinv[:], zinv[:], ZCORR)
                nc.vector.reciprocal(zinv[:], zinv[:])
                vts_tiles = []
                for tl in range(4):
                    t = 4 * g + tl
                    vts = vtspool.tile([128, M96], B16)
                    nc.vector.tensor_scalar_mul(
                        vts[:], vt[:, M96 * t:M96 * (t + 1)],
                        zinv[:, tl:tl + 1])
                    vts_tiles.append(vts)
                nxt = []
                for tl in range(4):
                    # interleave next group's QK chunks with this group's AV
                    # at 2-MM granularity so a psum-starved QK never leaves
                    # the in-order PE without ready AV work right behind it
                    es2 = None
                    if g < 7:
                        es2 = epool.tile([128, N], B16, name="es", tag="es")
                        nxt.append(es2)
                    pos = [opsum.tile([M96, 512], F32, tag="pos",
                                      name=f"pos{jj}")
                           for jj in range(2)]
                    for sl in range(4):
                        for jj in range(2):
                            j = 2 * tl + jj
                            nc.tensor.matmul(
                                pos[jj][:], lhsT=vts_tiles[sl],
                                rhs=strips[sl][:, 512 * j:512 * (j + 1)],
                                start=(sl == 0), stop=(sl == 3))
                        if es2 is not None:
                            qk_chunk(es2, 4 * (g + 1) + tl, sl)
                    for jj in range(2):
                        j = 2 * tl + jj
                        cols = slice(512 * j, 512 * (j + 1))
                        if g == 0:
                            nc.vector.tensor_copy(
                                out_acc[:, cols], pos[jj][:])
                        else:
                            nc.vector.tensor_add(
                                out_acc[:, cols], out_acc[:, cols],
                                pos[jj][:])
                        if g == 7:
                            # stripe j is final: stage + ship both halves
                            nc.vector.tensor_copy(
                                out_acc_r[:, cols], out_acc[:, cols])
                            nc.sync.dma_start(
                                cc_inA[CH * j:CH * (j + 1), :].rearrange(
                                    "cl (khw i w) -> (cl khw) i w",
                                    khw=8, i=8),
                                out_acc_r[0:48, cols].rearrange(
                                    "p (i w) -> p i w", i=8))
                            nc.scalar.dma_start(
                                cc_inB[CH * j:CH * (j + 1), :].rearrange(
                                    "cl (khw i w) -> (cl khw) i w",
                                    khw=8, i=8),
                                out_acc_r[48:96, cols].rearrange(
                                    "p (i w) -> p i w", i=8))
                strips = nxt

        # ---------------- split AllToAll + projection ----------------
        nc.gpsimd.collective_compute(
            "AllToAll", mybir.AluOpType.bypass,
            replica_groups=[list(range(NCORES))],
            ins=[cc_inA[:]], outs=[cc_outA[:]])
        nc.gpsimd.collective_compute(
            "AllToAll", mybir.AluOpType.bypass,
            replica_groups=[list(range(NCORES))],
            ins=[cc_inB[:]], outs=[cc_outB[:]])
        with tc.tile_pool(name="prj", bufs=4) as prj, \
             tc.tile_pool(name="prj_ps", bufs=3, space="PSUM") as prjps, \
             tc.tile_pool(name="yt", bufs=3) as ypool:
            for q in range(16):
                src = cc_outA if q < 8 else cc_outB
                lcols = slice(512 * (q % 8), 512 * (q % 8 + 1))
                fold_t = prj.tile([C, 512], B16)
                nc.sync.dma_start(fold_t[:], src[:, lcols])
                pp = prjps.tile([C, 512], F32)
                nc.tensor.matmul(pp[:], lhsT=wpt_sb[:], rhs=fold_t[:],
                                 start=True, stop=True)
                yt = ypool.tile([C, 512], F32)
                nc.vector.tensor_copy(yt[:], pp[:])
                nc.scalar.dma_start(y_h[:, 512 * q:512 * (q + 1)], yt[:])

    _skip_redundant_ldweights(nc)
    _split_excess_waits(nc)
    return nc


_ldwskip_ctr = [0]

def _skip_redundant_ldweights(nc):
    """An InstLdweights whose stationary operand matches the weights already
    sitting in the PE array (loaded by the previous InstLdweights, with only
    non-transpose matmuls in between) is redundant: the array state is
    unchanged. Convert it to a NoOp that keeps its sync_info."""
    import bass_rust
    import concourse.mybir as mybir
    PE = mybir.EngineType.PE

    def wsig(w):
        return (w.memref, w.offset, str(w.ap), str(w.dtype))

    for fn in nc.m.functions:
        for bb in fn.blocks:
            last = None
            out = []
            changed = False
            for inst in bb.instructions:
                if getattr(inst, "engine", None) == PE:
                    tn = type(inst).__name__
                    if tn == "InstLdweights":
                        s = wsig(inst.ins[0])
                        if s == last:
                            _ldwskip_ctr[0] += 1
                            nop = bass_rust.InstNoOp(
                                name=f"I-ldwskip-{_ldwskip_ctr[0]}",
                                ins=[], outs=[])
                            nop.engine = PE
                            nop.sync_info = inst.sync_info
                            out.append(nop)
                            changed = True
                            continue
                        last = s
                    elif tn == "InstMatmult":
                        if inst.is_transpose:
                            last = None
                    elif tn not in ("InstNoOp", "InstEventSemaphore"):
                        last = None
                out.append(inst)
            if changed:
                bb.instructions = out


_wsplit_ctr = [0]

def _split_excess_waits(nc, max_waits=1):
    """This walrus build encodes only one sync-wait per instruction; hoist
    extras onto same-engine nops inserted directly before the instruction."""
    import bass_rust
    import concourse.mybir as mybir
    for fn in nc.m.functions:
        for bb in fn.blocks:
            insts = bb.instructions
            out = []
            changed = False
            for inst in insts:
                si = inst.sync_info
                if si is not None and len(si.on_wait) > max_waits:
                    waits = list(si.on_wait)
                    for w in waits[:-max_waits]:
                        _wsplit_ctr[0] += 1
                        nop = bass_rust.InstNoOp(
                            name=f"I-wsplit-{_wsplit_ctr[0]}", ins=[], outs=[])
                        nop.engine = inst.engine
                        nop.sync_info = mybir.SyncInfo(
                            on_wait=[w], on_update=[])
                        out.append(nop)
                    inst.sync_info = mybir.SyncInfo(
                        on_wait=waits[-max_waits:],
                        on_update=list(si.on_update))
                    changed = True
                out.append(inst)
            if changed:
                bb.instructions = out


def _get_program():
    global _PROG
    if _PROG is None:
        _PROG = _build_program()
    return _PROG


# ---------------------------------------------------------------- entry

def kernel(x, w_qkv, w_dw, temperature, w_proj, _trace=False):
    x = np.asarray(x, np.float32)
    w_qkv = np.asarray(w_qkv, np.float32)
    w_dw = np.asarray(w_dw, np.float32)
    temperature = np.asarray(temperature, np.float32)
    w_proj = np.asarray(w_proj, np.float32)

    nc = _get_program()
    from concourse.bass_utils import run_bass_kernel_spmd

    xb = _build_xb(x)
    id96 = np.eye(M96, dtype=np.float32)
    wpt = np.ascontiguousarray(w_proj.T).astype(BF16)
    in_maps = []
    for h in range(NH):
        t_h = float(temperature[h, 0, 0])
        vcol = np.empty((M96, 2), np.float32)
        vcol[:, 0] = 1.0 / (t_h * t_h)
        vcol[:, 1] = 1.0
        in_maps.append({
            "xb": xb,
            "w4": _build_w4(h, w_qkv, w_dw),
            "vcol": vcol.astype(BF16),
            "wpt": wpt,
            "id96": id96,
            "onesrow": np.ones((1, M96), np.float32),
            "onesb": np.ones((1, M96), BF16),
        })

    res = run_bass_kernel_spmd(nc, in_maps, list(range(NCORES)), trace=_trace)

    y = np.empty((1, C, 256, 256), np.float32)
    for s in range(NCORES):
        blk = res.results[s]["y"].reshape(C, 4, 4, 8, GN)
        y[0, :, 32 * s:32 * (s + 1), :] = (
            blk.transpose(0, 3, 1, 4, 2).reshape(C, 32, 256))
    if _trace:
        return y, res
    return y


# revision 36
# speedup vs baseline: 1.2296x; 1.0598x over previous
"""Trainium2 Bass kernel for nn_AttentionV4 (patch attention, 8 heads on 8 cores).

Pipeline per core (= per head h), bf16 compute / fp32 accumulation:
  - The 1x1 qkv conv + depthwise 3x3 conv are fused into one dense 3x3 conv,
    expressed as a single matmul over a 6x6-windowed patch basis:
      Q/K/V[r, n] = sum_kappa W4[kappa, r] * Xp[kappa, n],
    kappa = (ph, pw, c) in [6,6,48] (1728, chunked 14 x 128), n = interior
    patch (64x64 grid = 4096; boundary patches of the stride-4 pad-4 unfold
    are exactly zero and are handled analytically). Xp chunks are DMA'd one
    per 128-kappa chunk via class-affine views of xb (each (dh,dw) class is
    an affine (hm, wm, c) box).
  - Head-matrix rows are ordered (khB, cl, kh1, kw) so each half of the fold
    rows (kh<2 vs kh>=2) is partition-contiguous; the output AllToAll splits
    into two halves, the second overlapping the first half's projection.
  - l2-normalize Q (x temperature) and K per column, A = Qn^T Kn in [-1,1],
    so softmax needs no max subtraction: E = exp(A), Z = rowsum(E) + 260
    (260 = number of zero boundary K columns, each contributing exp(0)).
  - out = (V/Z) @ E; attention is software-pipelined: QK strips of group g+1
    interleave with AV j-block pairs of group g so the PE never waits on exp.
"""
import sys
import types

sys.path.insert(0, "/opt/trn_rl_repo")

import numpy as np
import ml_dtypes

BF16 = ml_dtypes.bfloat16

# ---------------------------------------------------------------- constants
C = 48          # image channels
CH = 6          # channels per head
NH = 8          # heads == cores
GN = 64         # interior patch grid
N = GN * GN     # 4096 interior patches
M96 = 96        # rows of a head matrix (6ch * 4 * 4)
NKAP = 1728     # 36 windows * 48 channels
ZCORR = 260.0   # 4356 - 4096 zero K-columns, exp(0) each
NPIECE = 8      # front-end N pieces (8 patch rows, 512 patches each)
NCORES = 8
NCHUNK14 = 14

# group list (ph, pw) in kappa order
_GROUPS = []
for _dh, _dw in [(0, 0), (0, 1), (1, 0), (1, 1)]:
    for _hm in range(4 if _dh == 0 else 2):
        for _wm in range(4 if _dw == 0 else 2):
            _GROUPS.append((_dh * 4 + _hm, _dw * 4 + _wm))

# (dh, dw) class of each 128-kappa chunk (class boundaries at 768/1152/1536;
# chunk 13's pad rows 1728..1792 are genuine zeros, so its class is harmless)
def _chunk_plan14():
    cls_edges = [(0, (0, 0)), (768, (0, 1)), (1152, (1, 0)), (1536, (1, 1))]
    plan = []
    for k in range(NCHUNK14):
        k0 = 128 * k
        dh, dw = [c for e, c in cls_edges if e <= k0][-1]
        plan.append((dh, dw))
    return plan

CHUNKS14 = _chunk_plan14()

# kappa order (ph, pw, c), c fastest within each group
_PHS = np.repeat([g[0] for g in _GROUPS], C)
_PWS = np.repeat([g[1] for g in _GROUPS], C)
_CS = np.tile(np.arange(C), NKAP // C)

# ---------------------------------------------------------------- host prep

def _build_xb(x):
    """Kappa-major windowed image: xb2[kappa, hq, wq] (class-chunk order,
    padded to 14*128 rows so every front-end chunk is one affine DMA)."""
    xpad = np.zeros((C, 260, 260), np.float32)
    xpad[:, 1:257, 1:257] = x[0]
    xb = np.ascontiguousarray(
        xpad.reshape(C, 65, 4, 65, 4).transpose(2, 4, 0, 1, 3))
    xb2 = np.zeros((128 * NCHUNK14, 65, 65), np.float32)
    xb2[:NKAP] = xb[_PHS % 4, _PWS % 4, _CS]
    return xb2.astype(BF16)


def _build_w4(h, w_qkv, w_dw):
    """Fused (1x1 conv + dw3x3) weights in the kappa basis: [1792, 288].

    Output rows within a head matrix are ordered (khB, cl, kh1, kw) so the
    two fold halves (kh<2, kh>=2) are partition-contiguous."""
    kh = np.arange(4)
    dy = _PHS[:, None] - kh[None, :]            # [1728, 4]
    dx = _PWS[:, None] - kh[None, :]
    my = (dy >= 0) & (dy < 3)
    mx = (dx >= 0) & (dx < 3)
    dyc = np.clip(dy, 0, 2)
    dxc = np.clip(dx, 0, 2)
    w4 = np.zeros((NKAP, 3, CH, 4, 4), np.float32)
    for sel in range(3):
        for cl in range(CH):
            o = sel * C + CH * h + cl
            wd = w_dw[o, 0]
            taps = (wd[dyc[:, :, None], dxc[:, None, :]]
                    * my[:, :, None] * mx[:, None, :])
            w4[:, sel, cl] = w_qkv[o, _CS][:, None, None] * taps
    w4 = (w4.reshape(NKAP, 3, CH, 2, 2, 4).transpose(0, 1, 3, 2, 4, 5)
          .reshape(NKAP, 288))
    w4p = np.zeros((128 * NCHUNK14, 288), np.float32)
    w4p[:NKAP] = w4
    return w4p.astype(BF16)


# ---------------------------------------------------------------- program

_PROG = None

def _build_program():
    import antenv  # noqa: F401
    if "antenv.axon_hooks" not in sys.modules:
        holder = {}
        m = types.ModuleType("antenv.axon_hooks")
        m.set_axon_ntff_profile_hook = lambda hk: holder.__setitem__("h", hk)
        m.get_axon_ntff_profile_hook = lambda: holder.get("h")
        sys.modules["antenv.axon_hooks"] = m
        antenv.axon_hooks = m
        try:
            from trn_agent_boot.trn_boot import _ntff_profile_via_ctypes
            m.set_axon_ntff_profile_hook(
                _ntff_profile_via_ctypes("/opt/axon/libaxon_pjrt.so"))
        except Exception:
            pass

    import concourse.bass as bass
    import concourse.tile as tile
    import concourse.mybir as mybir
    from contextlib import ExitStack

    F32 = mybir.dt.float32
    B16 = mybir.dt.bfloat16
    AF = mybir.ActivationFunctionType

    nc = bass.Bass("TRN2", num_devices=NCORES)

    xb_h = nc.dram_tensor("xb", [128 * NCHUNK14, 65, 65], B16,
                          kind="ExternalInput")
    w4_h = nc.dram_tensor("w4", [128 * NCHUNK14, 288], B16,
                          kind="ExternalInput")
    vcol_h = nc.dram_tensor("vcol", [M96, 2], B16, kind="ExternalInput")
    wpt_h = nc.dram_tensor("wpt", [C, C], B16, kind="ExternalInput")
    id96_h = nc.dram_tensor("id96", [M96, M96], F32, kind="ExternalInput")
    ones_h = nc.dram_tensor("onesrow", [1, M96], F32, kind="ExternalInput")
    onesb_h = nc.dram_tensor("onesb", [1, M96], B16, kind="ExternalInput")
    y_h = nc.dram_tensor("y", [C, 8192], F32, kind="ExternalOutput")
    cc_inA = nc.dram_tensor("cc_inA", [C, 4096], B16)
    cc_outA = nc.dram_tensor("cc_outA", [C, 4096], B16)
    cc_inB = nc.dram_tensor("cc_inB", [C, 4096], B16)
    cc_outB = nc.dram_tensor("cc_outB", [C, 4096], B16)

    with tile.TileContext(nc) as tc, ExitStack() as ctx, \
            nc.allow_low_precision(reason="bf16 compute, fp32 accumulation"):
        const = ctx.enter_context(tc.tile_pool(name="const", bufs=1))
        w4_sb = const.tile([128, NCHUNK14, 288], B16)
        for k in range(NCHUNK14):
            nc.gpsimd.dma_start(w4_sb[:, k, :],
                                w4_h[128 * k:128 * (k + 1), :])
        vcol_sb = const.tile([M96, 2], B16)
        nc.gpsimd.dma_start(vcol_sb[:], vcol_h[:])
        wpt_sb = const.tile([C, C], B16)
        nc.gpsimd.dma_start(wpt_sb[:], wpt_h[:])
        id96_sb = const.tile([M96, M96], F32)
        nc.gpsimd.dma_start(id96_sb[:], id96_h[:])
        ones_sb = const.tile([1, M96], F32)
        nc.gpsimd.dma_start(ones_sb[:], ones_h[:])
        onesb_sb = const.tile([1, M96], B16)
        nc.gpsimd.dma_start(onesb_sb[:], onesb_h[:])

        persist = ctx.enter_context(tc.tile_pool(name="persist", bufs=1))
        qn = persist.tile([M96, N], B16)
        kn = persist.tile([M96, N], B16)
        vt = persist.tile([128, 32 * M96], B16)
        zacc = persist.tile([128, 128], F32)
        rqt = persist.tile([128, 32], F32)

        # attention SBUF pools allocated ahead of the front-end pools so the
        # E-strips don't inherit anti-dependencies from recycled FE memory
        epool = ctx.enter_context(tc.tile_pool(name="e_sb", bufs=10))
        zpool = ctx.enter_context(tc.tile_pool(name="z_sb", bufs=2))
        vtspool = ctx.enter_context(tc.tile_pool(name="vts", bufs=8))

        # ---------------- front end: Q/K/V + column sumsq ----------------
        ctx2 = tc.tile_pool(name="fe_persist", bufs=1)
        fep = ctx2.__enter__()
        vn = fep.tile([M96, N], F32)
        rq_row = fep.tile([1, N], F32)
        rk_row = fep.tile([1, N], B16)
        ph_ps_cm = tc.tile_pool(name="ph1_ps", bufs=1, space="PSUM")
        ph1ps = ph_ps_cm.__enter__()
        rqps = ph1ps.tile([128, 32], F32, bufs=1)
        with tc.tile_pool(name="fe_xp", bufs=2) as xp_pool, \
             tc.tile_pool(name="fe_tmp", bufs=2) as fe_tmp:
            for p in range(NPIECE):
                r0 = 8 * p
                xp_t = xp_pool.tile([128, NCHUNK14, 9, 65], B16,
                                    name="xp", tag="xp")
                for k2 in range(NCHUNK14 // 2):
                    nc.sync.dma_start(
                        xp_t[:, 2 * k2:2 * (k2 + 1), :, :],
                        xb_h[256 * k2:256 * (k2 + 1), r0:r0 + 9, :]
                        .rearrange("(a p) r w -> p a r w", p=128))
                cols = slice(512 * p, 512 * (p + 1))
                for sel, dst in ((0, qn), (1, kn), (2, vn)):
                    pss = ph1ps.tile([M96, 512], F32, name="pss",
                                     tag="ps", bufs=3)
                    for k, (dh, dw) in enumerate(CHUNKS14):
                        nc.tensor.matmul(
                            pss[:],
                            lhsT=w4_sb[:, k, M96 * sel:M96 * (sel + 1)],
                            rhs=xp_t[:, k, dh:dh + 8, dw:dw + 64],
                            start=(k == 0), stop=(k == NCHUNK14 - 1))
                    nc.vector.tensor_copy(dst[:, cols], pss[:])
                    if sel < 2:
                        sq = fe_tmp.tile([M96, 512], B16, name="sq", tag="sq")
                        nc.scalar.activation(sq[:], pss[:], AF.Square)
                        ssp = ph1ps.tile([1, 512], F32, name="ssp",
                                         tag="ssp", bufs=2)
                        nc.tensor.matmul(
                            ssp[:], lhsT=vcol_sb[:, sel:sel + 1], rhs=sq[:],
                            start=True, stop=True)
                        if sel == 0:
                            nc.vector.tensor_copy(rq_row[0:1, cols], ssp[:])
                            for i in range(4):
                                t = 4 * p + i
                                nc.tensor.transpose(
                                    rqps[:, t:t + 1],
                                    rq_row[0:1, 128 * t:128 * (t + 1)],
                                    ones_sb[0:1, 0:1])
                        else:
                            nc.vector.tensor_copy(rk_row[0:1, cols], ssp[:])
                            # normalize this piece's K columns immediately
                            bp = ph1ps.tile([M96, 512], F32, name="bp",
                                            tag="bp", bufs=2)
                            nc.tensor.matmul(bp[:], lhsT=onesb_sb[:],
                                             rhs=rk_row[0:1, cols],
                                             start=True, stop=True)
                            b = fe_tmp.tile([M96, 512], F32, name="b",
                                            tag="b")
                            nc.vector.reciprocal(b[:], bp[:])
                            brt = fe_tmp.tile([M96, 512], B16, name="brt",
                                              tag="brt")
                            nc.scalar.activation(brt[:], b[:], AF.Sqrt)
                            nc.vector.tensor_mul(kn[:, cols], kn[:, cols],
                                                 brt[:])

        # ---------------- rqt = rsqrt(sumsq_q) ----------------
        if True:
            nc.vector.reciprocal(rqt[:], rqps[:])
            nc.scalar.activation(rqt[:], rqt[:], AF.Sqrt)

        # ---------------- V^T via PE transpose ----------------
        if True:
            for t in range(32):
                tp = ph1ps.tile([128, M96], F32, name="tp", tag="ps", bufs=3)
                nc.tensor.transpose(
                    tp[:], vn[:, 128 * t:128 * (t + 1)], id96_sb[:])
                nc.vector.tensor_copy(vt[:, M96 * t:M96 * (t + 1)], tp[:])
        ph_ps_cm.__exit__(None, None, None)
        ctx2.__exit__(None, None, None)
        late = ctx.enter_context(tc.tile_pool(name="late", bufs=1))
        out_acc = late.tile([M96, N], F32)
        out_acc_r = late.tile([M96, N], B16)

        # ---------------- attention (software-pipelined) ----------------
        with tc.tile_pool(name="a_ps", bufs=3, space="PSUM") as apsum, \
             tc.tile_pool(name="o_ps", bufs=2, space="PSUM") as opsum:

            def qk_chunk(es, t, mp):
                # one [128, 1024] A-psum chunk + its exp
                pa = apsum.tile([128, 1024], F32, name="pa", tag="pa",
                                bufs=3)
                for half in range(2):
                    nc.tensor.matmul(
                        pa[:, 512 * half:512 * (half + 1)],
                        lhsT=qn[:, 128 * t:128 * (t + 1)],
                        rhs=kn[:, 1024 * mp + 512 * half:
                               1024 * mp + 512 * (half + 1)],
                        start=True, stop=True)
                col = 4 * t + mp
                nc.scalar.activation(
                    es[:, 1024 * mp:1024 * (mp + 1)], pa[:], AF.Exp,
                    scale=rqt[:, t:t + 1],
                    accum_out=zacc[:, col:col + 1])

            def qk_strip(g, tl):
                t = 4 * g + tl
                es = epool.tile([128, N], B16, name="es", tag="es")
                for mp in range(4):
                    qk_chunk(es, t, mp)
                return es

            strips = [qk_strip(0, tl) for tl in range(4)]
            for g in range(8):
                # Z for the group's 4 row-tiles: sum 4 accum cols, +260, 1/x
                zinv = zpool.tile([128, 4], F32)
                nc.vector.tensor_reduce(
                    zinv[:],
                    zacc[:, 16 * g:16 * (g + 1)].rearrange(
                        "p (t m) -> p t m", t=4),
                    axis=mybir.AxisListType.X, op=mybir.AluOpType.add)
                nc.vector.tensor_scalar_add(zinv[:], zinv[:], ZCORR)
                nc.vector.reciprocal(zinv[:], zinv[:])
                vts_tiles = []
                for tl in range(4):
                    t = 4 * g + tl
                    vts = vtspool.tile([128, M96], B16)
                    nc.vector.tensor_scalar_mul(
                        vts[:], vt[:, M96 * t:M96 * (t + 1)],
                        zinv[:, tl:tl + 1])
                    vts_tiles.append(vts)
                nxt = []
                for tl in range(4):
                    # interleave next group's QK chunks with this group's AV
                    # at 2-MM granularity so a psum-starved QK never leaves
                    # the in-order PE without ready AV work right behind it
                    es2 = None
                    if g < 7:
                        es2 = epool.tile([128, N], B16, name="es", tag="es")
                        nxt.append(es2)
                    pos = [opsum.tile([M96, 512], F32, tag="pos",
                                      name=f"pos{jj}")
                           for jj in range(2)]
                    for sl in range(4):
                        for jj in range(2):
                            j = 2 * tl + jj
                            nc.tensor.matmul(
                                pos[jj][:], lhsT=vts_tiles[sl],
                                rhs=strips[sl][:, 512 * j:512 * (j + 1)],
                                start=(sl == 0), stop=(sl == 3))
                        if es2 is not None:
                            qk_chunk(es2, 4 * (g + 1) + tl, sl)
                    for jj in range(2):
                        j = 2 * tl + jj
                        cols = slice(512 * j, 512 * (j + 1))
                        if g == 0:
                            nc.vector.tensor_copy(
                                out_acc[:, cols], pos[jj][:])
                        else:
                            nc.vector.tensor_add(
                                out_acc[:, cols], out_acc[:, cols],
                                pos[jj][:])
                        if g == 7:
                            # stripe j is final: stage + ship both halves
                            nc.vector.tensor_copy(
                                out_acc_r[:, cols], out_acc[:, cols])
                            nc.sync.dma_start(
                                cc_inA[CH * j:CH * (j + 1), :].rearrange(
                                    "cl (khw i w) -> (cl khw) i w",
                                    khw=8, i=8),
                                out_acc_r[0:48, cols].rearrange(
                                    "p (i w) -> p i w", i=8))
                            nc.scalar.dma_start(
                                cc_inB[CH * j:CH * (j + 1), :].rearrange(
                                    "cl (khw i w) -> (cl khw) i w",
                                    khw=8, i=8),
                                out_acc_r[48:96, cols].rearrange(
                                    "p (i w) -> p i w", i=8))
                strips = nxt

        # ---------------- split AllToAll + projection ----------------
        nc.gpsimd.collective_compute(
            "AllToAll", mybir.AluOpType.bypass,
            replica_groups=[list(range(NCORES))],
            ins=[cc_inA[:]], outs=[cc_outA[:]])
        nc.gpsimd.collective_compute(
            "AllToAll", mybir.AluOpType.bypass,
            replica_groups=[list(range(NCORES))],
            ins=[cc_inB[:]], outs=[cc_outB[:]])
        with tc.tile_pool(name="prj", bufs=4) as prj, \
             tc.tile_pool(name="prj_ps", bufs=3, space="PSUM") as prjps, \
             tc.tile_pool(name="yt", bufs=3) as ypool:
            for q in range(16):
                src = cc_outA if q < 8 else cc_outB
                lcols = slice(512 * (q % 8), 512 * (q % 8 + 1))
                fold_t = prj.tile([C, 512], B16)
                nc.sync.dma_start(fold_t[:], src[:, lcols])
                pp = prjps.tile([C, 512], F32)
                nc.tensor.matmul(pp[:], lhsT=wpt_sb[:], rhs=fold_t[:],
                                 start=True, stop=True)
                yt = ypool.tile([C, 512], F32)
                nc.vector.tensor_copy(yt[:], pp[:])
                nc.scalar.dma_start(y_h[:, 512 * q:512 * (q + 1)], yt[:])

    _skip_redundant_ldweights(nc)
    _split_excess_waits(nc)
    return nc


_ldwskip_ctr = [0]

def _skip_redundant_ldweights(nc):
    """An InstLdweights whose stationary operand matches the weights already
    sitting in the PE array (loaded by the previous InstLdweights, with only
    non-transpose matmuls in between) is redundant: the array state is
    unchanged. Convert it to a NoOp that keeps its sync_info."""
    import bass_rust
    import concourse.mybir as mybir
    PE = mybir.EngineType.PE

    def wsig(w):
        return (w.memref, w.offset, str(w.ap), str(w.dtype))

    for fn in nc.m.functions:
        for bb in fn.blocks:
            last = None
            out = []
            changed = False
            for inst in bb.instructions:
                if getattr(inst, "engine", None) == PE:
                    tn = type(inst).__name__
                    if tn == "InstLdweights":
                        s = wsig(inst.ins[0])
                        if s == last:
                            _ldwskip_ctr[0] += 1
                            nop = bass_rust.InstNoOp(
                                name=f"I-ldwskip-{_ldwskip_ctr[0]}",
                                ins=[], outs=[])
                            nop.engine = PE
                            nop.sync_info = inst.sync_info
                            out.append(nop)
                            changed = True
                            continue
                        last = s
                    elif tn == "InstMatmult":
                        if inst.is_transpose:
                            last = None
                    elif tn not in ("InstNoOp", "InstEventSemaphore"):
                        last = None
                out.append(inst)
            if changed:
                bb.instructions = out


_wsplit_ctr = [0]

def _split_excess_waits(nc, max_waits=1):
    """This walrus build encodes only one sync-wait per instruction; hoist
    extras onto same-engine nops inserted directly before the instruction."""
    import bass_rust
    import concourse.mybir as mybir
    for fn in nc.m.functions:
        for bb in fn.blocks:
            insts = bb.instructions
            out = []
            changed = False
            for inst in insts:
                si = inst.sync_info
                if si is not None and len(si.on_wait) > max_waits:
                    waits = list(si.on_wait)
                    for w in waits[:-max_waits]:
                        _wsplit_ctr[0] += 1
                        nop = bass_rust.InstNoOp(
                            name=f"I-wsplit-{_wsplit_ctr[0]}", ins=[], outs=[])
                        nop.engine = inst.engine
                        nop.sync_info = mybir.SyncInfo(
                            on_wait=[w], on_update=[])
                        out.append(nop)
                    inst.sync_info = mybir.SyncInfo(
                        on_wait=waits[-max_waits:],
                        on_update=list(si.on_update))
                    changed = True
                out.append(inst)
            if changed:
                bb.instructions = out


def _get_program():
    global _PROG
    if _PROG is None:
        _PROG = _build_program()
    return _PROG


# ---------------------------------------------------------------- entry

def kernel(x, w_qkv, w_dw, temperature, w_proj, _trace=False):
    x = np.asarray(x, np.float32)
    w_qkv = np.asarray(w_qkv, np.float32)
    w_dw = np.asarray(w_dw, np.float32)
    temperature = np.asarray(temperature, np.float32)
    w_proj = np.asarray(w_proj, np.float32)

    nc = _get_program()
    from concourse.bass_utils import run_bass_kernel_spmd

    xb = _build_xb(x)
    id96 = np.eye(M96, dtype=np.float32)
    wpt = np.ascontiguousarray(w_proj.T).astype(BF16)
    in_maps = []
    for h in range(NH):
        t_h = float(temperature[h, 0, 0])
        vcol = np.empty((M96, 2), np.float32)
        vcol[:, 0] = 1.0 / (t_h * t_h)
        vcol[:, 1] = 1.0
        in_maps.append({
            "xb": xb,
            "w4": _build_w4(h, w_qkv, w_dw),
            "vcol": vcol.astype(BF16),
            "wpt": wpt,
            "id96": id96,
            "onesrow": np.ones((1, M96), np.float32),
            "onesb": np.ones((1, M96), BF16),
        })

    res = run_bass_kernel_spmd(nc, in_maps, list(range(NCORES)), trace=_trace)

    y = np.empty((1, C, 256, 256), np.float32)
    for s in range(NCORES):
        blk = res.results[s]["y"].reshape(C, 4, 4, 8, GN)
        y[0, :, 32 * s:32 * (s + 1), :] = (
            blk.transpose(0, 3, 1, 4, 2).reshape(C, 32, 256))
    if _trace:
        return y, res
    return y
